# revision 35
# baseline (speedup 1.0000x reference)
"""GAT layer (dense-adj variant) on 8 Trainium2 NeuronCores — v4.

Row-parallel over destination nodes. Same math as v2/v3 (relu'd scores,
bias epilogue). v4 changes, all driven by the v3 trace (146.5us):

1. CONTIGUOUS DMAS: v3's xc loads were strided (512B rows, ~91GB/s per
   transfer, 4.3us for 256KB) and paced the whole B phase; the contiguous
   adj loads ran at ~400GB/s. v4 pre-tiles xT / rhs_aug / w_src on the
   HOST into the exact SBUF tile byte layout, so every transfer is
   contiguous: one 512KB xc DMA per 4-strip group (~1.3us) instead of two
   strided 256KB ones (~3-4us each), and half the sync-queue issue load.

2. B in 4-strip groups over two 2-bank psum double-tiles, drained by two
   2-block ACT copies (one per double-tile); one accumulation chain per
   psum BANK (a start=True resets the whole bank's has_written bits).
   Phase A reads the first two xc group tiles (node order is rolled
   host-side so each core's own rows come first) and slots into the PE
   stream after group 1.

3. EXPs all deferred past B's copies on ACT (EXP_AT empty by default):
   psum recycling during B never waits behind a 3.7us EXP, B runs at the
   ~111ns matmul cadence, and the 16-EXP chain overlaps phase C, which it
   co-paces with PE. DVE runs dst-extract + zb/za one group behind B.
   adj quad q is issued at group q+2 (contiguous 256KB x4), throttled by
   the 3-deep adj pool, keeping the early HBM window for xc.

4. Tail: phase C's last TWO quads run bank-pair-outer so accumulators
   finish staggered ~2.7us earlier; phase D chains (recip, scale, +fc_b,
   store) drain under C's tail with out DMAs on the idle-by-then sync
   queue (v3's gpsimd out DMAs added a 7us gpsimd drain at exit).
"""

import numpy as np
import ml_dtypes

N = 8192
IN_DIM = 512
OUT_DIM = 256
NCORES = 8
R = N // NCORES  # 1024 rows per core
KT = IN_DIM // 128  # 4 k-tiles
JT = N // 128  # 64 j-strips
IT = R // 128  # 8 i-tiles per core
NG = JT // 4  # 16 four-strip groups (= quads)
HA = OUT_DIM + 1  # moving-operand width (h | one)
HS = OUT_DIM + 2  # h slot width (dst | h | one)
GW = 2048  # xc group tile width: 4 strips x 512 cols

# after which B-groups' copies to emit the next EXP on ACT (za-readiness paced)
EXP_AT = ()
# dummy gpsimd memsets queued before adj quad q's DMA issues (time-pacing)
ADJ_DUMMIES = {}
# quads whose four block-DMAs are spread out by small dummies (burst-smoothing)
ADJ_SPREAD = (0, 1, 2, 3, 4, 5, 6)

bf16 = ml_dtypes.bfloat16

_cache = {}


def _build():
    import concourse.tile as tile
    from concourse import bacc, mybir

    AF = mybir.ActivationFunctionType
    ALU = mybir.AluOpType
    f32 = mybir.dt.float32
    bft = mybir.dt.bfloat16

    nc = bacc.Bacc("TRN2", target_bir_lowering=False, debug=False)

    # all tensors below are node-rolled per core on the host (strips 0-7 are
    # the core's own rows) and pre-tiled so every DMA is contiguous.
    adjT_d = nc.dram_tensor("adjT", [N, R], bft, kind="ExternalInput").ap()
    # xTt[g] = contiguous [128, 2048] group tiles: col = hp*1024 + kt*256 + n
    xTt_d = nc.dram_tensor("xTt", [NG * 128, GW], bft, kind="ExternalInput").ap()
    # rhs_t: [128, KT*HA], col = kt*HA + c, rows = k % 128 (k-chunk-tiled)
    rhs_t_d = nc.dram_tensor("rhs_t", [128, KT * HA], bft, kind="ExternalInput").ap()
    w_src_t_d = nc.dram_tensor("w_src_t", [128, KT * 128], bft, kind="ExternalInput").ap()
    src_bias_d = nc.dram_tensor("src_bias", [128, 1], f32, kind="ExternalInput").ap()
    fcb_d = nc.dram_tensor("fcb", [128, OUT_DIM], f32, kind="ExternalInput").ap()
    out_d = nc.dram_tensor("out", [R, OUT_DIM], f32, kind="ExternalOutput").ap()

    xTt_g = xTt_d.rearrange("(v p) c -> v p c", p=128)

    with tile.TileContext(nc) as tc:
        with (
            tc.tile_pool(name="const", bufs=1) as cpool,
            tc.tile_pool(name="hpool", bufs=1) as hpool,
            tc.tile_pool(name="xstream", bufs=5) as xpool,
            tc.tile_pool(name="astream", bufs=3) as apool,
            tc.tile_pool(name="work", bufs=2) as wpool,
            tc.tile_pool(name="estream", bufs=8) as epool,
            tc.tile_pool(name="opool", bufs=3) as opool,
        ):
            # ---- startup constants: rhs first on sync (mm0 needs it), the
            # rest on the scalar queue ----
            rhs_aug_sb = cpool.tile([128, KT * HA], bft)
            nc.sync.dma_start(rhs_aug_sb[:], rhs_t_d)
            w_src_sb = cpool.tile([128, KT * 128], bft)
            nc.scalar.dma_start(w_src_sb[:], w_src_t_d)
            src_bias_sb = cpool.tile([128, 1], f32)
            nc.scalar.dma_start(src_bias_sb[:], src_bias_d)
            fcb_sb = cpool.tile([128, OUT_DIM], f32)
            nc.scalar.dma_start(fcb_sb[:], fcb_d)

            scratch = cpool.tile([128, 4096], bft)
            src_rep = cpool.tile([128, R], bft)
            h_sb = hpool.tile([128, JT * HS], bft)
            dst_sb = cpool.tile([128, JT], f32)
            # ones column (slot offset 257) for the softmax denominator
            nc.gpsimd.memset(
                h_sb[:].rearrange("p (j c) -> p j c", c=HS)[:, :, HS - 1 : HS], 1.0
            )

            ps_ab_cm = tc.tile_pool(name="ps_ab", bufs=4, space="PSUM")
            ps_ab = ps_ab_cm.__enter__()

            e_quads = [None] * NG
            xcs = [None] * NG
            za_tiles = [None] * NG
            adj_tiles = [None] * NG
            next_exp = 0

            def emit_exp(q):
                e4 = epool.tile([128, 4096], bft, name="e4")
                nc.scalar.activation(e4[:], za_tiles[q][:], AF.Exp)
                e_quads[q] = e4

            def emit_adj(q):
                # DVE memset writes one col per 1024-block: all four DMAs
                # WAW-gate on it, so adj q's transfer starts only when the
                # DVE elementwise stream reaches this point. Quads >= 3 are
                # additionally paced by dummy gpsimd memsets queued before
                # their issues (ADJ_DUMMIES): the adj stream mostly moves to
                # the post-B window where the bus is otherwise idle, keeping
                # B's window for the two xc queues.
                at = apool.tile([128, 4096], bft, name="adj")
                nc.vector.memset(
                    at[:].rearrange("p (s n) -> p s n", s=4)[:, :, 0:1], 0.0
                )
                for _ in range(ADJ_DUMMIES.get(q, 0)):
                    nc.gpsimd.memset(scratch[:], 0.0)
                for s in range(4):
                    nc.gpsimd.dma_start(
                        at[:, s * 1024 : (s + 1) * 1024],
                        adjT_d[(4 * q + s) * 128 : (4 * q + s + 1) * 128, :],
                    )
                    if q in ADJ_SPREAD and s < 3:
                        nc.gpsimd.memset(scratch[:, 0:2048], 0.0)
                adj_tiles[q] = at

            def emit_ew(g):
                # DVE: dst extract for group g's 4 strips, then the quad's
                # elementwise (zb per strip, za quad-wide)
                if adj_tiles[g] is None:
                    emit_adj(g)
                nc.vector.tensor_copy(
                    dst_sb[:, 4 * g : 4 * g + 4],
                    h_sb[:, 4 * g * HS : (4 * g + 4) * HS].rearrange(
                        "p (j c) -> p j c", c=HS
                    )[:, :, 0:1],
                )
                zb = wpool.tile([128, 4096], bft, name="zb", tag="zb")
                for s in range(4):
                    nc.vector.tensor_scalar(
                        zb[:, s * 1024 : (s + 1) * 1024],
                        src_rep[:],
                        dst_sb[:, 4 * g + s : 4 * g + s + 1],
                        0.0,
                        ALU.add,
                        ALU.max,
                    )
                # prefetch next quad's adj so its transfer overlaps za
                if g + 1 < NG and adj_tiles[g + 1] is None:
                    emit_adj(g + 1)
                za = wpool.tile([128, 4096], bft, name="za", tag="za")
                nc.vector.tensor_mul(za[:], zb[:], adj_tiles[g][:])
                za_tiles[g] = za

            # ---- fused A+B: groups of 4 strips across 4 banks ----
            for g in range(NG):
                # one contiguous 512KB group DMA on sync
                xc = xpool.tile([128, GW], bft, name="xc")
                nc.sync.dma_start(xc[:], xTt_g[g])
                xcs[g] = xc

                # B matmuls: one 4-bank quad-tile per group; one chain per bank
                pb = [
                    ps_ab.tile([128, 1024], f32, name="ps_b", tag="ps")
                    for _ in range(2)
                ]
                for kt in range(KT):
                    for s in range(4):
                        col = (s // 2) * 1024 + kt * 256 + (s % 2) * 128
                        nc.tensor.matmul(
                            pb[s // 2][:, (s % 2) * 512 : (s % 2) * 512 + HA],
                            xc[:, col : col + 128],
                            rhs_aug_sb[:, kt * HA : (kt + 1) * HA],
                            start=(kt == 0),
                            stop=(kt == KT - 1),
                        )

                # phase A after group 1: reads the two local xc group tiles
                if g == 1:
                    ps_a = [
                        ps_ab.tile([128, 1024], f32, name="ps_a", tag="ps")
                        for _ in range(2)
                    ]
                    for kt in range(KT):
                        for pr in range(4):
                            acol = (pr % 2) * 1024 + kt * 256
                            nc.tensor.matmul(
                                ps_a[pr // 2][:, (pr % 2) * 512 : (pr % 2) * 512 + 256],
                                w_src_sb[:, kt * 128 : (kt + 1) * 128],
                                xcs[pr // 2][:, acol : acol + 256],
                                start=(kt == 0),
                                stop=(kt == KT - 1),
                            )

                # drain the two double-tiles with one 2-block ACT copy each:
                # slot layout [dst | h | one], ones preset
                for hp in range(2):
                    nc.scalar.activation(
                        h_sb[:, (4 * g + 2 * hp) * HS : (4 * g + 2 * hp + 2) * HS]
                        .rearrange("p (j c) -> p j c", c=HS)[:, :, 0:HA],
                        pb[hp][:].rearrange("p (j c) -> p j c", c=512)[:, :, 0:HA],
                        AF.Identity,
                    )

                if g == 1:
                    # src_rep for the elementwise stream (bias folds b_src+b_dst)
                    for ch in range(2):
                        nc.scalar.activation(
                            src_rep[:, ch * 512 : (ch + 1) * 512].rearrange(
                                "p (j c) -> p j c", c=256
                            ),
                            ps_a[ch][:].rearrange("p (j c) -> p j c", c=512)[
                                :, :, 0:256
                            ],
                            AF.Identity,
                            bias=src_bias_sb[:],
                        )

                # DVE elementwise for quad g-1 (one-group lag so the first
                # zb is emitted after src_rep's writers)
                if g >= 1:
                    emit_ew(g - 1)

                if g in EXP_AT:
                    emit_exp(next_exp)
                    next_exp += 1

            emit_ew(NG - 1)
            # remaining EXPs back-to-back; they overlap phase C's matmuls
            while next_exp < NG:
                emit_exp(next_exp)
                next_exp += 1

            # ---- Phase C: consume E quads, 8-bank round-robin ----
            ps_ab_cm.__exit__(None, None, None)
            with tc.tile_pool(name="ps_acc", bufs=1, space="PSUM") as ps_acc:
                out_ps = {}
                for it in range(IT):
                    out_ps[it] = ps_acc.tile(
                        [128, HA], f32, name=f"acc{it}", tag=f"acc{it}"
                    )
                for q in range(NG):
                    e4 = e_quads[q]
                    if q < NG - 2:
                        order = [(s, it) for s in range(4) for it in range(IT)]
                    else:
                        # bank-pair-outer on the last two quads: accumulator
                        # banks finish staggered so phase D pipelines under
                        # the remaining matmuls
                        order = [
                            (s, 2 * itp + e)
                            for itp in range(IT // 2)
                            for s in range(4)
                            for e in range(2)
                        ]
                    for s, it in order:
                        jt = 4 * q + s
                        hj = h_sb[:, jt * HS + 1 : jt * HS + 1 + HA]
                        nc.tensor.matmul(
                            out_ps[it][:, 0:HA],
                            e4[:, s * 1024 + it * 128 : s * 1024 + (it + 1) * 128],
                            hj,
                            start=(jt == 0),
                            stop=(jt == JT - 1),
                        )

                # ---- Phase D: normalize rows (col 256 = Z), + fc_b, store ----
                for it in range(IT):
                    rz = opool.tile([128, 1], f32, tag="rz")
                    nc.vector.reciprocal(rz[:], out_ps[it][:, OUT_DIM : OUT_DIM + 1])
                    o = opool.tile([128, OUT_DIM], f32, tag="o")
                    nc.scalar.activation(
                        o[:], out_ps[it][:, 0:OUT_DIM], AF.Identity, scale=rz[:]
                    )
                    o2 = opool.tile([128, OUT_DIM], f32, tag="o2")
                    nc.vector.tensor_add(o2[:], o[:], fcb_sb[:])
                    nc.sync.dma_start(out_d[it * 128 : (it + 1) * 128, :], o2[:])

    nc.compile()
    return nc


def _prep_inputs(adj, x, fc_w, fc_b, attn_w, attn_b):
    fc_w = np.asarray(fc_w, np.float32)
    fc_b = np.asarray(fc_b, np.float32)
    attn_w = np.asarray(attn_w, np.float32)
    a_src = fc_w @ attn_w[:OUT_DIM]
    a_dst = fc_w @ attn_w[OUT_DIM:]
    b_src = float(fc_b @ attn_w[:OUT_DIM]) + float(attn_b)
    b_dst = float(fc_b @ attn_w[OUT_DIM:])

    xT = np.ascontiguousarray(np.asarray(x, np.float32).T).astype(bf16)  # [512, N]
    adjT = np.asarray(adj, np.float32).astype(bf16).T  # [N (src j), N (dest i)]
    rhs_aug = np.concatenate([a_dst[:, None], fc_w], axis=1).astype(bf16)  # [512, 257]
    # k-chunk-tiled constants: row = k % 128, col blocks per kt
    rhs_t = np.ascontiguousarray(
        rhs_aug.reshape(KT, 128, HA).transpose(1, 0, 2).reshape(128, KT * HA)
    )
    w_src_t = np.ascontiguousarray(
        np.tile(a_src.astype(bf16).reshape(KT, 128).T.reshape(128, KT, 1), (1, 1, 128)).reshape(
            128, KT * 128
        )
    )
    src_bias = np.full((128, 1), b_src + b_dst, np.float32)
    fcb = np.tile(fc_b[None, :], (128, 1)).astype(np.float32)

    in_maps = []
    for c in range(NCORES):
        # roll node order so this core's own rows are strips 0-7
        xr = np.roll(xT, -c * R, axis=1)  # [512, N]
        # group tile layout: xTt[g][p, hp*1024 + kt*256 + n] = xr[kt*128+p, g*512+hp*256+n]
        xt4 = xr.reshape(KT, 128, NG, 2, 256)  # [kt, p, g, hp, n]
        xTt = np.ascontiguousarray(xt4.transpose(2, 1, 3, 0, 4)).reshape(NG * 128, GW)
        in_maps.append(
            {
                "adjT": np.ascontiguousarray(
                    np.roll(adjT[:, c * R : (c + 1) * R], -c * R, axis=0)
                ),
                "xTt": xTt,
                "rhs_t": rhs_t,
                "w_src_t": w_src_t,
                "src_bias": src_bias,
                "fcb": fcb,
            }
        )
    return in_maps


def kernel(adj, x, fc_w, fc_b, attn_w, attn_b, _trace=False, _tmpdir=None):
    from concourse import bass_utils

    if "nc" not in _cache:
        _cache["nc"] = _build()
    nc = _cache["nc"]
    in_maps = _prep_inputs(adj, x, fc_w, fc_b, attn_w, attn_b)
    res = bass_utils.run_bass_kernel_spmd(
        nc,
        in_maps,
        core_ids=list(range(NCORES)),
        trace=_trace,
        **({"tmpdir": _tmpdir} if _tmpdir else {}),
    )
    out = np.concatenate([res.results[c]["out"] for c in range(NCORES)], axis=0)
    if _trace:
        _cache["last_exec_time_ns"] = res.exec_time_ns
        _cache["last_profile_json"] = res.profile_json
    return out


# revision 36
# speedup vs baseline: 1.0315x; 1.0315x over previous
"""GAT layer (dense-adj variant) on 8 Trainium2 NeuronCores — final (v4 config).

Row-parallel over destination nodes; same math as the 148us baseline
(relu'd edge scores, softmax-invariant bias epilogue), rebuilt around the
measured engine/DMA behavior. HW exec ~130-131us (baseline 148.7).

Key structure:
- CONTIGUOUS DMAS: xT / rhs_aug / w_src are pre-tiled on the HOST into
  exact SBUF tile byte layouts; every transfer is contiguous (strided xc
  loads ran at ~91GB/s and paced all of phase B; contiguous runs ~400).
  One 512KB xc DMA per 4-strip group on the sync queue; adj quads are
  4x256KB on gpsimd, WAW-gated behind a DVE memset inside the elementwise
  stream so they self-pace and leave the early bus to xc.
- No xTi input: each core's node order is rolled host-side (own rows
  first), so phase A (src replication) reads B's first two xc tiles and
  slots into the PE stream after group 1. One accumulation chain per PSUM
  BANK everywhere (a start=True resets the whole bank's has_written bits).
- B: 4-strip groups across 4 banks (two 2-bank double-tiles, ring of 4),
  ~111ns matmul cadence; each double-tile drains with one 2-block ACT
  copy, so psum recycling never waits on DVE. DVE runs dst-extract +
  zb/za one group behind B; EXPs are emitted after the loop and the Tile
  scheduler hoists 4-5 of them into B's DMA slack, the rest overlapping C.
- C: 512 matmuls round-robin over 8 accumulator banks, PE-dense at 111ns;
  the last two quads run bank-pair-outer so accumulators finish staggered
  and phase D (recip, scale, +fc_b, store on sync) drains under C's tail.

Measured dead ends (do not revisit without new evidence): fp8 anywhere
(DoubleRow is fp8-only and LDWEIGHTS-bound at 257-col moving width; fp8
operands drop DVE tensor_tensor to 1x; fp8 h breaks the 2e-2 gate),
splitting xc across the scalar queue (its DMA issues poison the in-order
ACT queue and stall the psum-freeing copies), epool<=3 EXP throttles
(starve C via ring round-trip latency), explicit EXP placement or adj
burst-smoothing dummies (the scheduler's own placement beats both).
"""

import numpy as np
import ml_dtypes

N = 8192
IN_DIM = 512
OUT_DIM = 256
NCORES = 8
R = N // NCORES  # 1024 rows per core
KT = IN_DIM // 128  # 4 k-tiles
JT = N // 128  # 64 j-strips
IT = R // 128  # 8 i-tiles per core
NG = JT // 4  # 16 four-strip groups (= quads)
HA = OUT_DIM + 1  # moving-operand width (h | one)
HS = OUT_DIM + 2  # h slot width (dst | h | one)
GW = 2048  # xc group tile width: 4 strips x 512 cols

# after which B-groups' copies to emit the next EXP on ACT (za-readiness paced)
EXP_AT = ()
# dummy gpsimd memsets queued before adj quad q's DMA issues (time-pacing)
ADJ_DUMMIES = {}
# quads whose four block-DMAs are spread out by small dummies (burst-smoothing)
ADJ_SPREAD = ()

bf16 = ml_dtypes.bfloat16

_cache = {}


def _build():
    import concourse.tile as tile
    from concourse import bacc, mybir

    AF = mybir.ActivationFunctionType
    ALU = mybir.AluOpType
    f32 = mybir.dt.float32
    bft = mybir.dt.bfloat16

    nc = bacc.Bacc("TRN2", target_bir_lowering=False, debug=False)

    # all tensors below are node-rolled per core on the host (strips 0-7 are
    # the core's own rows) and pre-tiled so every DMA is contiguous.
    adjT_d = nc.dram_tensor("adjT", [N, R], bft, kind="ExternalInput").ap()
    # xTt[g] = contiguous [128, 2048] group tiles: col = hp*1024 + kt*256 + n
    xTt_d = nc.dram_tensor("xTt", [NG * 128, GW], bft, kind="ExternalInput").ap()
    # rhs_t: [128, KT*HA], col = kt*HA + c, rows = k % 128 (k-chunk-tiled)
    rhs_t_d = nc.dram_tensor("rhs_t", [128, KT * HA], bft, kind="ExternalInput").ap()
    w_src_t_d = nc.dram_tensor("w_src_t", [128, KT * 128], bft, kind="ExternalInput").ap()
    src_bias_d = nc.dram_tensor("src_bias", [128, 1], f32, kind="ExternalInput").ap()
    fcb_d = nc.dram_tensor("fcb", [128, OUT_DIM], f32, kind="ExternalInput").ap()
    out_d = nc.dram_tensor("out", [R, OUT_DIM], f32, kind="ExternalOutput").ap()

    xTt_g = xTt_d.rearrange("(v p) c -> v p c", p=128)

    with tile.TileContext(nc) as tc:
        with (
            tc.tile_pool(name="const", bufs=1) as cpool,
            tc.tile_pool(name="hpool", bufs=1) as hpool,
            tc.tile_pool(name="xstream", bufs=5) as xpool,
            tc.tile_pool(name="astream", bufs=3) as apool,
            tc.tile_pool(name="work", bufs=2) as wpool,
            tc.tile_pool(name="estream", bufs=8) as epool,
            tc.tile_pool(name="opool", bufs=3) as opool,
        ):
            # ---- startup constants: rhs first on sync (mm0 needs it), the
            # rest on the scalar queue ----
            rhs_aug_sb = cpool.tile([128, KT * HA], bft)
            nc.sync.dma_start(rhs_aug_sb[:], rhs_t_d)
            w_src_sb = cpool.tile([128, KT * 128], bft)
            nc.scalar.dma_start(w_src_sb[:], w_src_t_d)
            src_bias_sb = cpool.tile([128, 1], f32)
            nc.scalar.dma_start(src_bias_sb[:], src_bias_d)
            fcb_sb = cpool.tile([128, OUT_DIM], f32)
            nc.scalar.dma_start(fcb_sb[:], fcb_d)

            scratch = cpool.tile([128, 4096], bft)
            src_rep = cpool.tile([128, R], bft)
            h_sb = hpool.tile([128, JT * HS], bft)
            dst_sb = cpool.tile([128, JT], f32)
            # ones column (slot offset 257) for the softmax denominator
            nc.gpsimd.memset(
                h_sb[:].rearrange("p (j c) -> p j c", c=HS)[:, :, HS - 1 : HS], 1.0
            )

            ps_ab_cm = tc.tile_pool(name="ps_ab", bufs=4, space="PSUM")
            ps_ab = ps_ab_cm.__enter__()

            e_quads = [None] * NG
            xcs = [None] * NG
            za_tiles = [None] * NG
            adj_tiles = [None] * NG
            next_exp = 0

            def emit_exp(q):
                e4 = epool.tile([128, 4096], bft, name="e4")
                nc.scalar.activation(e4[:], za_tiles[q][:], AF.Exp)
                e_quads[q] = e4

            def emit_adj(q):
                # DVE memset writes one col per 1024-block: all four DMAs
                # WAW-gate on it, so adj q's transfer starts only when the
                # DVE elementwise stream reaches this point. Quads >= 3 are
                # additionally paced by dummy gpsimd memsets queued before
                # their issues (ADJ_DUMMIES): the adj stream mostly moves to
                # the post-B window where the bus is otherwise idle, keeping
                # B's window for the two xc queues.
                at = apool.tile([128, 4096], bft, name="adj")
                nc.vector.memset(
                    at[:].rearrange("p (s n) -> p s n", s=4)[:, :, 0:1], 0.0
                )
                for _ in range(ADJ_DUMMIES.get(q, 0)):
                    nc.gpsimd.memset(scratch[:], 0.0)
                for s in range(4):
                    nc.gpsimd.dma_start(
                        at[:, s * 1024 : (s + 1) * 1024],
                        adjT_d[(4 * q + s) * 128 : (4 * q + s + 1) * 128, :],
                    )
                    if q in ADJ_SPREAD and s < 3:
                        nc.gpsimd.memset(scratch[:, 0:2048], 0.0)
                adj_tiles[q] = at

            def emit_ew(g):
                # DVE: dst extract for group g's 4 strips, then the quad's
                # elementwise (zb per strip, za quad-wide)
                if adj_tiles[g] is None:
                    emit_adj(g)
                nc.vector.tensor_copy(
                    dst_sb[:, 4 * g : 4 * g + 4],
                    h_sb[:, 4 * g * HS : (4 * g + 4) * HS].rearrange(
                        "p (j c) -> p j c", c=HS
                    )[:, :, 0:1],
                )
                zb = wpool.tile([128, 4096], bft, name="zb", tag="zb")
                for s in range(4):
                    nc.vector.tensor_scalar(
                        zb[:, s * 1024 : (s + 1) * 1024],
                        src_rep[:],
                        dst_sb[:, 4 * g + s : 4 * g + s + 1],
                        0.0,
                        ALU.add,
                        ALU.max,
                    )
                # prefetch next quad's adj so its transfer overlaps za
                if g + 1 < NG and adj_tiles[g + 1] is None:
                    emit_adj(g + 1)
                za = wpool.tile([128, 4096], bft, name="za", tag="za")
                nc.vector.tensor_mul(za[:], zb[:], adj_tiles[g][:])
                za_tiles[g] = za

            # ---- fused A+B: groups of 4 strips across 4 banks ----
            for g in range(NG):
                # one contiguous 512KB group DMA on sync
                xc = xpool.tile([128, GW], bft, name="xc")
                nc.sync.dma_start(xc[:], xTt_g[g])
                xcs[g] = xc

                # B matmuls: one 4-bank quad-tile per group; one chain per bank
                pb = [
                    ps_ab.tile([128, 1024], f32, name="ps_b", tag="ps")
                    for _ in range(2)
                ]
                for kt in range(KT):
                    for s in range(4):
                        col = (s // 2) * 1024 + kt * 256 + (s % 2) * 128
                        nc.tensor.matmul(
                            pb[s // 2][:, (s % 2) * 512 : (s % 2) * 512 + HA],
                            xc[:, col : col + 128],
                            rhs_aug_sb[:, kt * HA : (kt + 1) * HA],
                            start=(kt == 0),
                            stop=(kt == KT - 1),
                        )

                # phase A after group 1: reads the two local xc group tiles
                if g == 1:
                    ps_a = [
                        ps_ab.tile([128, 1024], f32, name="ps_a", tag="ps")
                        for _ in range(2)
                    ]
                    for kt in range(KT):
                        for pr in range(4):
                            acol = (pr % 2) * 1024 + kt * 256
                            nc.tensor.matmul(
                                ps_a[pr // 2][:, (pr % 2) * 512 : (pr % 2) * 512 + 256],
                                w_src_sb[:, kt * 128 : (kt + 1) * 128],
                                xcs[pr // 2][:, acol : acol + 256],
                                start=(kt == 0),
                                stop=(kt == KT - 1),
                            )

                # drain the two double-tiles with one 2-block ACT copy each:
                # slot layout [dst | h | one], ones preset
                for hp in range(2):
                    nc.scalar.activation(
                        h_sb[:, (4 * g + 2 * hp) * HS : (4 * g + 2 * hp + 2) * HS]
                        .rearrange("p (j c) -> p j c", c=HS)[:, :, 0:HA],
                        pb[hp][:].rearrange("p (j c) -> p j c", c=512)[:, :, 0:HA],
                        AF.Identity,
                    )

                if g == 1:
                    # src_rep for the elementwise stream (bias folds b_src+b_dst)
                    for ch in range(2):
                        nc.scalar.activation(
                            src_rep[:, ch * 512 : (ch + 1) * 512].rearrange(
                                "p (j c) -> p j c", c=256
                            ),
                            ps_a[ch][:].rearrange("p (j c) -> p j c", c=512)[
                                :, :, 0:256
                            ],
                            AF.Identity,
                            bias=src_bias_sb[:],
                        )

                # DVE elementwise for quad g-1 (one-group lag so the first
                # zb is emitted after src_rep's writers)
                if g >= 1:
                    emit_ew(g - 1)

                if g in EXP_AT:
                    emit_exp(next_exp)
                    next_exp += 1

            emit_ew(NG - 1)
            # remaining EXPs back-to-back; they overlap phase C's matmuls
            while next_exp < NG:
                emit_exp(next_exp)
                next_exp += 1

            # ---- Phase C: consume E quads, 8-bank round-robin ----
            ps_ab_cm.__exit__(None, None, None)
            with tc.tile_pool(name="ps_acc", bufs=1, space="PSUM") as ps_acc:
                out_ps = {}
                for it in range(IT):
                    out_ps[it] = ps_acc.tile(
                        [128, HA], f32, name=f"acc{it}", tag=f"acc{it}"
                    )
                for q in range(NG):
                    e4 = e_quads[q]
                    if q < NG - 2:
                        order = [(s, it) for s in range(4) for it in range(IT)]
                    else:
                        # bank-pair-outer on the last two quads: accumulator
                        # banks finish staggered so phase D pipelines under
                        # the remaining matmuls
                        order = [
                            (s, 2 * itp + e)
                            for itp in range(IT // 2)
                            for s in range(4)
                            for e in range(2)
                        ]
                    for s, it in order:
                        jt = 4 * q + s
                        hj = h_sb[:, jt * HS + 1 : jt * HS + 1 + HA]
                        nc.tensor.matmul(
                            out_ps[it][:, 0:HA],
                            e4[:, s * 1024 + it * 128 : s * 1024 + (it + 1) * 128],
                            hj,
                            start=(jt == 0),
                            stop=(jt == JT - 1),
                        )

                # ---- Phase D: normalize rows (col 256 = Z), + fc_b, store ----
                for it in range(IT):
                    rz = opool.tile([128, 1], f32, tag="rz")
                    nc.vector.reciprocal(rz[:], out_ps[it][:, OUT_DIM : OUT_DIM + 1])
                    o = opool.tile([128, OUT_DIM], f32, tag="o")
                    nc.scalar.activation(
                        o[:], out_ps[it][:, 0:OUT_DIM], AF.Identity, scale=rz[:]
                    )
                    o2 = opool.tile([128, OUT_DIM], f32, tag="o2")
                    nc.vector.tensor_add(o2[:], o[:], fcb_sb[:])
                    nc.sync.dma_start(out_d[it * 128 : (it + 1) * 128, :], o2[:])

    nc.compile()
    return nc


def _prep_inputs(adj, x, fc_w, fc_b, attn_w, attn_b):
    fc_w = np.asarray(fc_w, np.float32)
    fc_b = np.asarray(fc_b, np.float32)
    attn_w = np.asarray(attn_w, np.float32)
    a_src = fc_w @ attn_w[:OUT_DIM]
    a_dst = fc_w @ attn_w[OUT_DIM:]
    b_src = float(fc_b @ attn_w[:OUT_DIM]) + float(attn_b)
    b_dst = float(fc_b @ attn_w[OUT_DIM:])

    xT = np.ascontiguousarray(np.asarray(x, np.float32).T).astype(bf16)  # [512, N]
    adjT = np.asarray(adj, np.float32).astype(bf16).T  # [N (src j), N (dest i)]
    rhs_aug = np.concatenate([a_dst[:, None], fc_w], axis=1).astype(bf16)  # [512, 257]
    # k-chunk-tiled constants: row = k % 128, col blocks per kt
    rhs_t = np.ascontiguousarray(
        rhs_aug.reshape(KT, 128, HA).transpose(1, 0, 2).reshape(128, KT * HA)
    )
    w_src_t = np.ascontiguousarray(
        np.tile(a_src.astype(bf16).reshape(KT, 128).T.reshape(128, KT, 1), (1, 1, 128)).reshape(
            128, KT * 128
        )
    )
    src_bias = np.full((128, 1), b_src + b_dst, np.float32)
    fcb = np.tile(fc_b[None, :], (128, 1)).astype(np.float32)

    in_maps = []
    for c in range(NCORES):
        # roll node order so this core's own rows are strips 0-7
        xr = np.roll(xT, -c * R, axis=1)  # [512, N]
        # group tile layout: xTt[g][p, hp*1024 + kt*256 + n] = xr[kt*128+p, g*512+hp*256+n]
        xt4 = xr.reshape(KT, 128, NG, 2, 256)  # [kt, p, g, hp, n]
        xTt = np.ascontiguousarray(xt4.transpose(2, 1, 3, 0, 4)).reshape(NG * 128, GW)
        in_maps.append(
            {
                "adjT": np.ascontiguousarray(
                    np.roll(adjT[:, c * R : (c + 1) * R], -c * R, axis=0)
                ),
                "xTt": xTt,
                "rhs_t": rhs_t,
                "w_src_t": w_src_t,
                "src_bias": src_bias,
                "fcb": fcb,
            }
        )
    return in_maps


def kernel(adj, x, fc_w, fc_b, attn_w, attn_b, _trace=False, _tmpdir=None):
    from concourse import bass_utils

    if "nc" not in _cache:
        _cache["nc"] = _build()
    nc = _cache["nc"]
    in_maps = _prep_inputs(adj, x, fc_w, fc_b, attn_w, attn_b)
    res = bass_utils.run_bass_kernel_spmd(
        nc,
        in_maps,
        core_ids=list(range(NCORES)),
        trace=_trace,
        **({"tmpdir": _tmpdir} if _tmpdir else {}),
    )
    out = np.concatenate([res.results[c]["out"] for c in range(NCORES)], axis=0)
    if _trace:
        _cache["last_exec_time_ns"] = res.exec_time_ns
        _cache["last_profile_json"] = res.profile_json
    return out


# revision 38
# speedup vs baseline: 1.0449x; 1.0130x over previous
"""GAT layer (dense-adj variant) on 8 Trainium2 NeuronCores — final (v4 config).

Row-parallel over destination nodes; same math as the 148us baseline
(relu'd edge scores, softmax-invariant bias epilogue), rebuilt around the
measured engine/DMA behavior. HW exec ~130-131us (baseline 148.7).

Key structure:
- CONTIGUOUS DMAS: xT / rhs_aug / w_src are pre-tiled on the HOST into
  exact SBUF tile byte layouts; every transfer is contiguous (strided xc
  loads ran at ~91GB/s and paced all of phase B; contiguous runs ~400).
  One 512KB xc DMA per 4-strip group on the sync queue; adj quads are
  4x256KB on gpsimd, WAW-gated behind a DVE memset inside the elementwise
  stream so they self-pace and leave the early bus to xc.
- No xTi input: each core's node order is rolled host-side (own rows
  first), so phase A (src replication) reads B's first two xc tiles and
  slots into the PE stream after group 1. One accumulation chain per PSUM
  BANK everywhere (a start=True resets the whole bank's has_written bits).
- B: 4-strip groups across 4 banks (two 2-bank double-tiles, ring of 4),
  ~111ns matmul cadence; each double-tile drains with one 2-block ACT
  copy, so psum recycling never waits on DVE. DVE runs dst-extract +
  zb/za one group behind B; EXPs are emitted after the loop and the Tile
  scheduler hoists 4-5 of them into B's DMA slack, the rest overlapping C.
- C: 512 matmuls round-robin over 8 accumulator banks, PE-dense at 111ns;
  the last two quads run bank-pair-outer so accumulators finish staggered
  and phase D (recip, scale, +fc_b, store on sync) drains under C's tail.

Measured dead ends (do not revisit without new evidence): fp8 anywhere
(DoubleRow is fp8-only and LDWEIGHTS-bound at 257-col moving width; fp8
operands drop DVE tensor_tensor to 1x; fp8 h breaks the 2e-2 gate),
splitting xc across the scalar queue (its DMA issues poison the in-order
ACT queue and stall the psum-freeing copies), epool<=3 EXP throttles
(starve C via ring round-trip latency), explicit EXP placement or adj
burst-smoothing dummies (the scheduler's own placement beats both).
"""

import numpy as np
import ml_dtypes

N = 8192
IN_DIM = 512
OUT_DIM = 256
NCORES = 8
R = N // NCORES  # 1024 rows per core
KT = IN_DIM // 128  # 4 k-tiles
JT = N // 128  # 64 j-strips
IT = R // 128  # 8 i-tiles per core
NG = JT // 4  # 16 four-strip groups (= quads)
HA = OUT_DIM + 1  # moving-operand width (h | one)
HS = OUT_DIM + 2  # h slot width (dst | h | one)
GW = 2048  # xc group tile width: 4 strips x 512 cols

# after which B-groups' copies to emit the next EXP on ACT (za-readiness paced)
EXP_AT = ()
# dummy gpsimd memsets queued before adj quad q's DMA issues (time-pacing)
ADJ_DUMMIES = {}
# quads whose four block-DMAs are spread out by small dummies (burst-smoothing)
ADJ_SPREAD = ()

bf16 = ml_dtypes.bfloat16

_cache = {}


def _build():
    import concourse.tile as tile
    from concourse import bacc, mybir

    AF = mybir.ActivationFunctionType
    ALU = mybir.AluOpType
    f32 = mybir.dt.float32
    bft = mybir.dt.bfloat16

    nc = bacc.Bacc("TRN2", target_bir_lowering=False, debug=False)

    # all tensors below are node-rolled per core on the host (strips 0-7 are
    # the core's own rows) and pre-tiled so every DMA is contiguous.
    adjT_d = nc.dram_tensor("adjT", [N, R], bft, kind="ExternalInput").ap()
    # xTt[g] = contiguous [128, 2048] group tiles: col = hp*1024 + kt*256 + n
    xTt_d = nc.dram_tensor("xTt", [NG * 128, GW], bft, kind="ExternalInput").ap()
    # rhs_t: [IN_DIM, HA] = [a_dst | fc_w] row-major (per-kt slices contiguous)
    rhs_t_d = nc.dram_tensor("rhs_t", [IN_DIM, HA], bft, kind="ExternalInput").ap()
    w_src_t_d = nc.dram_tensor("w_src_t", [128, KT * 128], bft, kind="ExternalInput").ap()
    src_bias_d = nc.dram_tensor("src_bias", [128, 1], f32, kind="ExternalInput").ap()
    fcb_d = nc.dram_tensor("fcb", [128, OUT_DIM], f32, kind="ExternalInput").ap()
    out_d = nc.dram_tensor("out", [R, OUT_DIM], f32, kind="ExternalOutput").ap()

    xTt_g = xTt_d.rearrange("(v p) c -> v p c", p=128)

    with tile.TileContext(nc) as tc:
        with (
            tc.tile_pool(name="const", bufs=1) as cpool,
            tc.tile_pool(name="hpool", bufs=1) as hpool,
            tc.tile_pool(name="xstream", bufs=5) as xpool,
            tc.tile_pool(name="astream", bufs=3) as apool,
            tc.tile_pool(name="work", bufs=2) as wpool,
            tc.tile_pool(name="estream", bufs=8) as epool,
            tc.tile_pool(name="opool", bufs=3) as opool,
        ):
            # ---- startup constants: rhs first on sync (mm0 needs it), the
            # rest on the scalar queue ----
            rhs_k = []
            for kt in range(KT):
                rk = cpool.tile([128, HA], bft, name=f"rhs{kt}", tag=f"rhs{kt}")
                nc.sync.dma_start(rk[:], rhs_t_d[kt * 128 : (kt + 1) * 128, :])
                rhs_k.append(rk)
            w_src_sb = cpool.tile([128, KT * 128], bft)
            nc.scalar.dma_start(w_src_sb[:], w_src_t_d)
            src_bias_sb = cpool.tile([128, 1], f32)
            nc.scalar.dma_start(src_bias_sb[:], src_bias_d)
            fcb_sb = cpool.tile([128, OUT_DIM], f32)
            nc.scalar.dma_start(fcb_sb[:], fcb_d)

            scratch = cpool.tile([128, 4096], bft)
            src_rep = cpool.tile([128, R], bft)
            h_sb = hpool.tile([128, JT * HS], bft)
            dst_sb = cpool.tile([128, JT], f32)
            # ones column (slot offset 257) for the softmax denominator
            nc.gpsimd.memset(
                h_sb[:].rearrange("p (j c) -> p j c", c=HS)[:, :, HS - 1 : HS], 1.0
            )

            ps_ab_cm = tc.tile_pool(name="ps_ab", bufs=4, space="PSUM")
            ps_ab = ps_ab_cm.__enter__()

            e_quads = [None] * NG
            xcs = [None] * NG
            za_tiles = [None] * NG
            adj_tiles = [None] * NG
            next_exp = 0

            def emit_exp(q):
                e4 = epool.tile([128, 4096], bft, name="e4")
                nc.scalar.activation(e4[:], za_tiles[q][:], AF.Exp)
                e_quads[q] = e4

            def emit_adj(q):
                # DVE memset writes one col per 1024-block: all four DMAs
                # WAW-gate on it, so adj q's transfer starts only when the
                # DVE elementwise stream reaches this point. Quads >= 3 are
                # additionally paced by dummy gpsimd memsets queued before
                # their issues (ADJ_DUMMIES): the adj stream mostly moves to
                # the post-B window where the bus is otherwise idle, keeping
                # B's window for the two xc queues.
                at = apool.tile([128, 4096], bft, name="adj")
                nc.vector.memset(
                    at[:].rearrange("p (s n) -> p s n", s=4)[:, :, 0:1], 0.0
                )
                for _ in range(ADJ_DUMMIES.get(q, 0)):
                    nc.gpsimd.memset(scratch[:], 0.0)
                for s in range(4):
                    nc.gpsimd.dma_start(
                        at[:, s * 1024 : (s + 1) * 1024],
                        adjT_d[(4 * q + s) * 128 : (4 * q + s + 1) * 128, :],
                    )
                    if q in ADJ_SPREAD and s < 3:
                        nc.gpsimd.memset(scratch[:, 0:2048], 0.0)
                adj_tiles[q] = at

            def emit_ew(g):
                # DVE: dst extract for group g's 4 strips, then the quad's
                # elementwise (zb per strip, za quad-wide)
                if adj_tiles[g] is None:
                    emit_adj(g)
                nc.vector.tensor_copy(
                    dst_sb[:, 4 * g : 4 * g + 4],
                    h_sb[:, 4 * g * HS : (4 * g + 4) * HS].rearrange(
                        "p (j c) -> p j c", c=HS
                    )[:, :, 0:1],
                )
                zb = wpool.tile([128, 4096], bft, name="zb", tag="zb")
                for s in range(4):
                    nc.vector.tensor_scalar(
                        zb[:, s * 1024 : (s + 1) * 1024],
                        src_rep[:],
                        dst_sb[:, 4 * g + s : 4 * g + s + 1],
                        0.0,
                        ALU.add,
                        ALU.max,
                    )
                # prefetch next quad's adj so its transfer overlaps za
                if g + 1 < NG and adj_tiles[g + 1] is None:
                    emit_adj(g + 1)
                za = wpool.tile([128, 4096], bft, name="za", tag="za")
                nc.vector.tensor_mul(za[:], zb[:], adj_tiles[g][:])
                za_tiles[g] = za

            # ---- fused A+B: groups of 4 strips across 4 banks ----
            for g in range(NG):
                # one contiguous 512KB group DMA on sync
                xc = xpool.tile([128, GW], bft, name="xc")
                nc.sync.dma_start(xc[:], xTt_g[g])
                xcs[g] = xc

                # B matmuls: one 4-bank quad-tile per group; one chain per bank
                pb = [
                    ps_ab.tile([128, 1024], f32, name="ps_b", tag="ps")
                    for _ in range(2)
                ]
                for kt in range(KT):
                    for s in range(4):
                        col = (s // 2) * 1024 + kt * 256 + (s % 2) * 128
                        nc.tensor.matmul(
                            pb[s // 2][:, (s % 2) * 512 : (s % 2) * 512 + HA],
                            xc[:, col : col + 128],
                            rhs_k[kt][:],
                            start=(kt == 0),
                            stop=(kt == KT - 1),
                        )

                # phase A after group 1: reads the two local xc group tiles
                if g == 1:
                    ps_a = [
                        ps_ab.tile([128, 1024], f32, name="ps_a", tag="ps")
                        for _ in range(2)
                    ]
                    for kt in range(KT):
                        for pr in range(4):
                            acol = (pr % 2) * 1024 + kt * 256
                            nc.tensor.matmul(
                                ps_a[pr // 2][:, (pr % 2) * 512 : (pr % 2) * 512 + 256],
                                w_src_sb[:, kt * 128 : (kt + 1) * 128],
                                xcs[pr // 2][:, acol : acol + 256],
                                start=(kt == 0),
                                stop=(kt == KT - 1),
                            )

                # drain the two double-tiles with one 2-block ACT copy each:
                # slot layout [dst | h | one], ones preset
                for hp in range(2):
                    nc.scalar.activation(
                        h_sb[:, (4 * g + 2 * hp) * HS : (4 * g + 2 * hp + 2) * HS]
                        .rearrange("p (j c) -> p j c", c=HS)[:, :, 0:HA],
                        pb[hp][:].rearrange("p (j c) -> p j c", c=512)[:, :, 0:HA],
                        AF.Identity,
                    )

                if g == 1:
                    # src_rep for the elementwise stream (bias folds b_src+b_dst)
                    for ch in range(2):
                        nc.scalar.activation(
                            src_rep[:, ch * 512 : (ch + 1) * 512].rearrange(
                                "p (j c) -> p j c", c=256
                            ),
                            ps_a[ch][:].rearrange("p (j c) -> p j c", c=512)[
                                :, :, 0:256
                            ],
                            AF.Identity,
                            bias=src_bias_sb[:],
                        )

                # DVE elementwise for quad g-1 (one-group lag so the first
                # zb is emitted after src_rep's writers)
                if g >= 1:
                    emit_ew(g - 1)

                if g in EXP_AT:
                    emit_exp(next_exp)
                    next_exp += 1

            emit_ew(NG - 1)
            # remaining EXPs back-to-back; they overlap phase C's matmuls
            while next_exp < NG:
                emit_exp(next_exp)
                next_exp += 1

            # ---- Phase C: consume E quads, 8-bank round-robin ----
            ps_ab_cm.__exit__(None, None, None)
            with tc.tile_pool(name="ps_acc", bufs=1, space="PSUM") as ps_acc:
                out_ps = {}
                for it in range(IT):
                    out_ps[it] = ps_acc.tile(
                        [128, HA], f32, name=f"acc{it}", tag=f"acc{it}"
                    )
                for q in range(NG):
                    e4 = e_quads[q]
                    if q < NG - 2:
                        order = [(s, it) for s in range(4) for it in range(IT)]
                    else:
                        # bank-pair-outer on the last two quads: accumulator
                        # banks finish staggered so phase D pipelines under
                        # the remaining matmuls
                        order = [
                            (s, 2 * itp + e)
                            for itp in range(IT // 2)
                            for s in range(4)
                            for e in range(2)
                        ]
                    for s, it in order:
                        jt = 4 * q + s
                        hj = h_sb[:, jt * HS + 1 : jt * HS + 1 + HA]
                        nc.tensor.matmul(
                            out_ps[it][:, 0:HA],
                            e4[:, s * 1024 + it * 128 : s * 1024 + (it + 1) * 128],
                            hj,
                            start=(jt == 0),
                            stop=(jt == JT - 1),
                        )

                # ---- Phase D: normalize rows (col 256 = Z), + fc_b, store ----
                for it in range(IT):
                    rz = opool.tile([128, 1], f32, tag="rz")
                    nc.vector.reciprocal(rz[:], out_ps[it][:, OUT_DIM : OUT_DIM + 1])
                    o = opool.tile([128, OUT_DIM], f32, tag="o")
                    nc.vector.tensor_scalar(
                        o[:], out_ps[it][:, 0:OUT_DIM], rz[:], None, ALU.mult
                    )
                    o2 = opool.tile([128, OUT_DIM], f32, tag="o2")
                    nc.vector.tensor_add(o2[:], o[:], fcb_sb[:])
                    nc.sync.dma_start(out_d[it * 128 : (it + 1) * 128, :], o2[:])

    nc.compile()
    return nc


def _prep_inputs(adj, x, fc_w, fc_b, attn_w, attn_b):
    fc_w = np.asarray(fc_w, np.float32)
    fc_b = np.asarray(fc_b, np.float32)
    attn_w = np.asarray(attn_w, np.float32)
    a_src = fc_w @ attn_w[:OUT_DIM]
    a_dst = fc_w @ attn_w[OUT_DIM:]
    b_src = float(fc_b @ attn_w[:OUT_DIM]) + float(attn_b)
    b_dst = float(fc_b @ attn_w[OUT_DIM:])

    xT = np.ascontiguousarray(np.asarray(x, np.float32).T).astype(bf16)  # [512, N]
    adjT = np.asarray(adj, np.float32).astype(bf16).T  # [N (src j), N (dest i)]
    rhs_aug = np.concatenate([a_dst[:, None], fc_w], axis=1).astype(bf16)  # [512, 257]
    # k-chunk-tiled constants: row = k % 128, col blocks per kt
    rhs_t = np.ascontiguousarray(rhs_aug)
    w_src_t = np.ascontiguousarray(
        np.tile(a_src.astype(bf16).reshape(KT, 128).T.reshape(128, KT, 1), (1, 1, 128)).reshape(
            128, KT * 128
        )
    )
    src_bias = np.full((128, 1), b_src + b_dst, np.float32)
    fcb = np.tile(fc_b[None, :], (128, 1)).astype(np.float32)

    in_maps = []
    for c in range(NCORES):
        # roll node order so this core's own rows are strips 0-7
        xr = np.roll(xT, -c * R, axis=1)  # [512, N]
        # group tile layout: xTt[g][p, hp*1024 + kt*256 + n] = xr[kt*128+p, g*512+hp*256+n]
        xt4 = xr.reshape(KT, 128, NG, 2, 256)  # [kt, p, g, hp, n]
        xTt = np.ascontiguousarray(xt4.transpose(2, 1, 3, 0, 4)).reshape(NG * 128, GW)
        in_maps.append(
            {
                "adjT": np.ascontiguousarray(
                    np.roll(adjT[:, c * R : (c + 1) * R], -c * R, axis=0)
                ),
                "xTt": xTt,
                "rhs_t": rhs_t,
                "w_src_t": w_src_t,
                "src_bias": src_bias,
                "fcb": fcb,
            }
        )
    return in_maps


def kernel(adj, x, fc_w, fc_b, attn_w, attn_b, _trace=False, _tmpdir=None):
    from concourse import bass_utils

    if "nc" not in _cache:
        _cache["nc"] = _build()
    nc = _cache["nc"]
    in_maps = _prep_inputs(adj, x, fc_w, fc_b, attn_w, attn_b)
    res = bass_utils.run_bass_kernel_spmd(
        nc,
        in_maps,
        core_ids=list(range(NCORES)),
        trace=_trace,
        **({"tmpdir": _tmpdir} if _tmpdir else {}),
    )
    out = np.concatenate([res.results[c]["out"] for c in range(NCORES)], axis=0)
    if _trace:
        _cache["last_exec_time_ns"] = res.exec_time_ns
        _cache["last_profile_json"] = res.profile_json
    return out


# revision 39
# speedup vs baseline: 1.0620x; 1.0164x over previous
"""GAT layer (dense-adj variant) on 8 Trainium2 NeuronCores — final (v4 config).

Row-parallel over destination nodes; same math as the 148us baseline
(relu'd edge scores, softmax-invariant bias epilogue), rebuilt around the
measured engine/DMA behavior. HW exec ~130-131us (baseline 148.7).

Key structure:
- CONTIGUOUS DMAS: xT / rhs_aug / w_src are pre-tiled on the HOST into
  exact SBUF tile byte layouts; every transfer is contiguous (strided xc
  loads ran at ~91GB/s and paced all of phase B; contiguous runs ~400).
  One 512KB xc DMA per 4-strip group on the sync queue; adj quads are
  4x256KB on gpsimd, WAW-gated behind a DVE memset inside the elementwise
  stream so they self-pace and leave the early bus to xc.
- No xTi input: each core's node order is rolled host-side (own rows
  first), so phase A (src replication) reads B's first two xc tiles and
  slots into the PE stream after group 1. One accumulation chain per PSUM
  BANK everywhere (a start=True resets the whole bank's has_written bits).
- B: 4-strip groups across 4 banks (two 2-bank double-tiles, ring of 4),
  ~111ns matmul cadence; each double-tile drains with one 2-block ACT
  copy, so psum recycling never waits on DVE. DVE runs dst-extract +
  zb/za one group behind B; EXPs are emitted after the loop and the Tile
  scheduler hoists 4-5 of them into B's DMA slack, the rest overlapping C.
- C: 512 matmuls round-robin over 8 accumulator banks, PE-dense at 111ns;
  the last two quads run bank-pair-outer so accumulators finish staggered
  and phase D (recip, scale, +fc_b, store on sync) drains under C's tail.

Measured dead ends (do not revisit without new evidence): fp8 anywhere
(DoubleRow is fp8-only and LDWEIGHTS-bound at 257-col moving width; fp8
operands drop DVE tensor_tensor to 1x; fp8 h breaks the 2e-2 gate),
splitting xc across the scalar queue (its DMA issues poison the in-order
ACT queue and stall the psum-freeing copies), epool<=3 EXP throttles
(starve C via ring round-trip latency), explicit EXP placement or adj
burst-smoothing dummies (the scheduler's own placement beats both).
"""

import numpy as np
import ml_dtypes

N = 8192
IN_DIM = 512
OUT_DIM = 256
NCORES = 8
R = N // NCORES  # 1024 rows per core
KT = IN_DIM // 128  # 4 k-tiles
JT = N // 128  # 64 j-strips
IT = R // 128  # 8 i-tiles per core
NG = JT // 4  # 16 four-strip groups (= quads)
HA = OUT_DIM + 1  # moving-operand width (h | one)
HS = OUT_DIM + 2  # h slot width (dst | h | one)
GW = 2048  # xc group tile width: 4 strips x 512 cols

# after which B-groups' copies to emit the next EXP on ACT (za-readiness paced)
EXP_AT = ()
# dummy gpsimd memsets queued before adj quad q's DMA issues (time-pacing)
ADJ_DUMMIES = {}
# quads whose four block-DMAs are spread out by small dummies (burst-smoothing)
ADJ_SPREAD = ()

bf16 = ml_dtypes.bfloat16

_cache = {}


def _build():
    import concourse.tile as tile
    from concourse import bacc, mybir

    AF = mybir.ActivationFunctionType
    ALU = mybir.AluOpType
    f32 = mybir.dt.float32
    bft = mybir.dt.bfloat16

    nc = bacc.Bacc("TRN2", target_bir_lowering=False, debug=False)

    # all tensors below are node-rolled per core on the host (strips 0-7 are
    # the core's own rows) and pre-tiled so every DMA is contiguous.
    adjT_d = nc.dram_tensor("adjT", [N, R], bft, kind="ExternalInput").ap()
    # xTt[g] = contiguous [128, 2048] group tiles: col = hp*1024 + kt*256 + n
    xTt_d = nc.dram_tensor("xTt", [NG * 128, GW], bft, kind="ExternalInput").ap()
    # rhs_t: [IN_DIM, HA] = [a_dst | fc_w] row-major (per-kt slices contiguous)
    rhs_t_d = nc.dram_tensor("rhs_t", [IN_DIM, HA], bft, kind="ExternalInput").ap()
    w_src_t_d = nc.dram_tensor("w_src_t", [128, KT * 128], bft, kind="ExternalInput").ap()
    src_bias_d = nc.dram_tensor("src_bias", [128, 1], f32, kind="ExternalInput").ap()
    fcb_d = nc.dram_tensor("fcb", [128, OUT_DIM], f32, kind="ExternalInput").ap()
    out_d = nc.dram_tensor("out", [R, OUT_DIM], f32, kind="ExternalOutput").ap()

    xTt_g = xTt_d.rearrange("(v p) c -> v p c", p=128)

    with tile.TileContext(nc) as tc:
        with (
            tc.tile_pool(name="const", bufs=1) as cpool,
            tc.tile_pool(name="hpool", bufs=1) as hpool,
            tc.tile_pool(name="xstream", bufs=8) as xpool,
            tc.tile_pool(name="astream", bufs=3) as apool,
            tc.tile_pool(name="work", bufs=2) as wpool,
            tc.tile_pool(name="estream", bufs=8) as epool,
            tc.tile_pool(name="opool", bufs=3) as opool,
        ):
            # ---- startup constants: rhs first on sync (mm0 needs it), the
            # rest on the scalar queue ----
            rhs_k = []
            for kt in range(KT):
                rk = cpool.tile([128, HA], bft, name=f"rhs{kt}", tag=f"rhs{kt}")
                nc.sync.dma_start(rk[:], rhs_t_d[kt * 128 : (kt + 1) * 128, :])
                rhs_k.append(rk)
            w_src_sb = cpool.tile([128, KT * 128], bft)
            nc.scalar.dma_start(w_src_sb[:], w_src_t_d)
            src_bias_sb = cpool.tile([128, 1], f32)
            nc.scalar.dma_start(src_bias_sb[:], src_bias_d)
            fcb_sb = cpool.tile([128, OUT_DIM], f32)
            nc.scalar.dma_start(fcb_sb[:], fcb_d)

            scratch = cpool.tile([128, 4096], bft)
            src_rep = cpool.tile([128, R], bft)
            h_sb = hpool.tile([128, JT * HS], bft)
            dst_sb = cpool.tile([128, JT], f32)
            # ones column (slot offset 257) for the softmax denominator
            nc.gpsimd.memset(
                h_sb[:].rearrange("p (j c) -> p j c", c=HS)[:, :, HS - 1 : HS], 1.0
            )

            ps_ab_cm = tc.tile_pool(name="ps_ab", bufs=4, space="PSUM")
            ps_ab = ps_ab_cm.__enter__()

            e_quads = [None] * NG
            xcs = [None] * NG
            za_tiles = [None] * NG
            adj_tiles = [None] * NG
            next_exp = 0

            def emit_exp(q):
                e4 = epool.tile([128, 4096], bft, name="e4")
                nc.scalar.activation(e4[:], za_tiles[q][:], AF.Exp)
                e_quads[q] = e4

            def emit_adj(q):
                # DVE memset writes one col per 1024-block: all four DMAs
                # WAW-gate on it, so adj q's transfer starts only when the
                # DVE elementwise stream reaches this point. Quads >= 3 are
                # additionally paced by dummy gpsimd memsets queued before
                # their issues (ADJ_DUMMIES): the adj stream mostly moves to
                # the post-B window where the bus is otherwise idle, keeping
                # B's window for the two xc queues.
                at = apool.tile([128, 4096], bft, name="adj")
                nc.vector.memset(
                    at[:].rearrange("p (s n) -> p s n", s=4)[:, :, 0:1], 0.0
                )
                for _ in range(ADJ_DUMMIES.get(q, 0)):
                    nc.gpsimd.memset(scratch[:], 0.0)
                for s in range(4):
                    nc.gpsimd.dma_start(
                        at[:, s * 1024 : (s + 1) * 1024],
                        adjT_d[(4 * q + s) * 128 : (4 * q + s + 1) * 128, :],
                    )
                    if q in ADJ_SPREAD and s < 3:
                        nc.gpsimd.memset(scratch[:, 0:2048], 0.0)
                adj_tiles[q] = at

            def emit_ew(g):
                # DVE: dst extract for group g's 4 strips, then the quad's
                # elementwise (zb per strip, za quad-wide)
                if adj_tiles[g] is None:
                    emit_adj(g)
                nc.vector.tensor_copy(
                    dst_sb[:, 4 * g : 4 * g + 4],
                    h_sb[:, 4 * g * HS : (4 * g + 4) * HS].rearrange(
                        "p (j c) -> p j c", c=HS
                    )[:, :, 0:1],
                )
                zb = wpool.tile([128, 4096], bft, name="zb", tag="zb")
                for s in range(4):
                    nc.vector.tensor_scalar(
                        zb[:, s * 1024 : (s + 1) * 1024],
                        src_rep[:],
                        dst_sb[:, 4 * g + s : 4 * g + s + 1],
                        0.0,
                        ALU.add,
                        ALU.max,
                    )
                # prefetch next quad's adj so its transfer overlaps za
                if g + 1 < NG and adj_tiles[g + 1] is None:
                    emit_adj(g + 1)
                za = wpool.tile([128, 4096], bft, name="za", tag="za")
                nc.vector.tensor_mul(za[:], zb[:], adj_tiles[g][:])
                za_tiles[g] = za

            # ---- fused A+B: groups of 4 strips across 4 banks ----
            for g in range(NG):
                # one contiguous 512KB group DMA on sync
                xc = xpool.tile([128, GW], bft, name="xc")
                nc.sync.dma_start(xc[:], xTt_g[g])
                xcs[g] = xc

                # B matmuls: one 4-bank quad-tile per group; one chain per bank
                pb = [
                    ps_ab.tile([128, 1024], f32, name="ps_b", tag="ps")
                    for _ in range(2)
                ]
                for kt in range(KT):
                    for s in range(4):
                        col = (s // 2) * 1024 + kt * 256 + (s % 2) * 128
                        nc.tensor.matmul(
                            pb[s // 2][:, (s % 2) * 512 : (s % 2) * 512 + HA],
                            xc[:, col : col + 128],
                            rhs_k[kt][:],
                            start=(kt == 0),
                            stop=(kt == KT - 1),
                        )

                # phase A after group 1: reads the two local xc group tiles
                if g == 1:
                    ps_a = [
                        ps_ab.tile([128, 1024], f32, name="ps_a", tag="ps")
                        for _ in range(2)
                    ]
                    for kt in range(KT):
                        for pr in range(4):
                            acol = (pr % 2) * 1024 + kt * 256
                            nc.tensor.matmul(
                                ps_a[pr // 2][:, (pr % 2) * 512 : (pr % 2) * 512 + 256],
                                w_src_sb[:, kt * 128 : (kt + 1) * 128],
                                xcs[pr // 2][:, acol : acol + 256],
                                start=(kt == 0),
                                stop=(kt == KT - 1),
                            )

                # drain the two double-tiles with one 2-block ACT copy each:
                # slot layout [dst | h | one], ones preset
                for hp in range(2):
                    nc.scalar.activation(
                        h_sb[:, (4 * g + 2 * hp) * HS : (4 * g + 2 * hp + 2) * HS]
                        .rearrange("p (j c) -> p j c", c=HS)[:, :, 0:HA],
                        pb[hp][:].rearrange("p (j c) -> p j c", c=512)[:, :, 0:HA],
                        AF.Identity,
                    )

                if g == 1:
                    # src_rep for the elementwise stream (bias folds b_src+b_dst)
                    for ch in range(2):
                        nc.scalar.activation(
                            src_rep[:, ch * 512 : (ch + 1) * 512].rearrange(
                                "p (j c) -> p j c", c=256
                            ),
                            ps_a[ch][:].rearrange("p (j c) -> p j c", c=512)[
                                :, :, 0:256
                            ],
                            AF.Identity,
                            bias=src_bias_sb[:],
                        )

                # DVE elementwise for quad g-1 (one-group lag so the first
                # zb is emitted after src_rep's writers)
                if g >= 1:
                    emit_ew(g - 1)

                if g in EXP_AT:
                    emit_exp(next_exp)
                    next_exp += 1

            emit_ew(NG - 1)
            # remaining EXPs back-to-back; they overlap phase C's matmuls
            while next_exp < NG:
                emit_exp(next_exp)
                next_exp += 1

            # ---- Phase C: consume E quads, 8-bank round-robin ----
            ps_ab_cm.__exit__(None, None, None)
            with tc.tile_pool(name="ps_acc", bufs=1, space="PSUM") as ps_acc:
                out_ps = {}
                for it in range(IT):
                    out_ps[it] = ps_acc.tile(
                        [128, HA], f32, name=f"acc{it}", tag=f"acc{it}"
                    )
                for q in range(NG):
                    e4 = e_quads[q]
                    if q < NG - 2:
                        order = [(s, it) for s in range(4) for it in range(IT)]
                    else:
                        # bank-pair-outer on the last two quads: accumulator
                        # banks finish staggered so phase D pipelines under
                        # the remaining matmuls
                        order = [
                            (s, 2 * itp + e)
                            for itp in range(IT // 2)
                            for s in range(4)
                            for e in range(2)
                        ]
                    for s, it in order:
                        jt = 4 * q + s
                        hj = h_sb[:, jt * HS + 1 : jt * HS + 1 + HA]
                        nc.tensor.matmul(
                            out_ps[it][:, 0:HA],
                            e4[:, s * 1024 + it * 128 : s * 1024 + (it + 1) * 128],
                            hj,
                            start=(jt == 0),
                            stop=(jt == JT - 1),
                        )

                # ---- Phase D: normalize rows (col 256 = Z), + fc_b, store ----
                for it in range(IT):
                    rz = opool.tile([128, 1], f32, tag="rz")
                    nc.vector.reciprocal(rz[:], out_ps[it][:, OUT_DIM : OUT_DIM + 1])
                    o = opool.tile([128, OUT_DIM], f32, tag="o")
                    nc.vector.tensor_scalar(
                        o[:], out_ps[it][:, 0:OUT_DIM], rz[:], None, ALU.mult
                    )
                    o2 = opool.tile([128, OUT_DIM], f32, tag="o2")
                    nc.vector.tensor_add(o2[:], o[:], fcb_sb[:])
                    nc.sync.dma_start(out_d[it * 128 : (it + 1) * 128, :], o2[:])

    nc.compile()
    return nc


def _prep_inputs(adj, x, fc_w, fc_b, attn_w, attn_b):
    fc_w = np.asarray(fc_w, np.float32)
    fc_b = np.asarray(fc_b, np.float32)
    attn_w = np.asarray(attn_w, np.float32)
    a_src = fc_w @ attn_w[:OUT_DIM]
    a_dst = fc_w @ attn_w[OUT_DIM:]
    b_src = float(fc_b @ attn_w[:OUT_DIM]) + float(attn_b)
    b_dst = float(fc_b @ attn_w[OUT_DIM:])

    xT = np.ascontiguousarray(np.asarray(x, np.float32).T).astype(bf16)  # [512, N]
    adjT = np.asarray(adj, np.float32).astype(bf16).T  # [N (src j), N (dest i)]
    rhs_aug = np.concatenate([a_dst[:, None], fc_w], axis=1).astype(bf16)  # [512, 257]
    # k-chunk-tiled constants: row = k % 128, col blocks per kt
    rhs_t = np.ascontiguousarray(rhs_aug)
    w_src_t = np.ascontiguousarray(
        np.tile(a_src.astype(bf16).reshape(KT, 128).T.reshape(128, KT, 1), (1, 1, 128)).reshape(
            128, KT * 128
        )
    )
    src_bias = np.full((128, 1), b_src + b_dst, np.float32)
    fcb = np.tile(fc_b[None, :], (128, 1)).astype(np.float32)

    in_maps = []
    for c in range(NCORES):
        # roll node order so this core's own rows are strips 0-7
        xr = np.roll(xT, -c * R, axis=1)  # [512, N]
        # group tile layout: xTt[g][p, hp*1024 + kt*256 + n] = xr[kt*128+p, g*512+hp*256+n]
        xt4 = xr.reshape(KT, 128, NG, 2, 256)  # [kt, p, g, hp, n]
        xTt = np.ascontiguousarray(xt4.transpose(2, 1, 3, 0, 4)).reshape(NG * 128, GW)
        in_maps.append(
            {
                "adjT": np.ascontiguousarray(
                    np.roll(adjT[:, c * R : (c + 1) * R], -c * R, axis=0)
                ),
                "xTt": xTt,
                "rhs_t": rhs_t,
                "w_src_t": w_src_t,
                "src_bias": src_bias,
                "fcb": fcb,
            }
        )
    return in_maps


def kernel(adj, x, fc_w, fc_b, attn_w, attn_b, _trace=False, _tmpdir=None):
    from concourse import bass_utils

    if "nc" not in _cache:
        _cache["nc"] = _build()
    nc = _cache["nc"]
    in_maps = _prep_inputs(adj, x, fc_w, fc_b, attn_w, attn_b)
    res = bass_utils.run_bass_kernel_spmd(
        nc,
        in_maps,
        core_ids=list(range(NCORES)),
        trace=_trace,
        **({"tmpdir": _tmpdir} if _tmpdir else {}),
    )
    out = np.concatenate([res.results[c]["out"] for c in range(NCORES)], axis=0)
    if _trace:
        _cache["last_exec_time_ns"] = res.exec_time_ns
        _cache["last_profile_json"] = res.profile_json
    return out
